# revision 5
# baseline (speedup 1.0000x reference)
"""BertSelfAttention Trainium2 Bass kernel.

Problem: B=8, L=1024, H=1024, 16 heads x 64 dim, fp32.
Sharding: data-parallel over batch -- one batch element per NeuronCore (8 cores).

Per-core algorithm (everything in "transposed" layout; host transposes in/out):
  inputs:  hT = hidden[b].T  [H, L] f32 (fed to PE as float32r)
           wqT/wkT/wvT = W.T [H, H] f32r, biases [1, H]
  1. v[j, dv] = sum_h hT[h, j] * wvT[h, dv] + bv   (PE, f32r)
       stored as vhat[j, head, 0:64] bf16 with vhat[.., 64] = 1.0 (ones column)
  2. per head-pair c (heads 2c, 2c+1 live in partitions 0:64 / 64:128 of chunk c):
       qT[dq, i], kT[dq, i]  (PE, f32r; bias via K=1 matmul with ones row)
       scoresT[j, i] = kT.T-slice @ qT-slice  -- two K=64 matmuls packed in the
         128-row PE array via tile_position=(64, 0)
       attnT = exp(SCALE * scoresT)  (ACT, PSUM->SBUF, bf16 out; no max-subtraction:
         scores ~ N(0,1), |s|<~6, exact in fp32)
       ctxT[d, i] (+ den in row 64) = vhat.T @ attnT  (PE, bf16, K=1024 accumulated;
         ones column of vhat yields softmax denominator for free)
       ctx = ctxT * (1/den) broadcast  (DVE recip + GpSimd partition_broadcast + DVE mul)
  Emission is software-pipelined: QK/exp of pair c interleaves with projections of
  pair c+1 and AV of pair c-1 so ACT exp time hides under PE work.

Output outT [H, L] per core; host takes outT.T -> ctx[b] [L, H].
"""

import numpy as np

import concourse.bacc as bacc
import concourse.mybir as mybir
import concourse.tile as tile
from concourse import bass_utils

B, L, H = 8, 1024, 1024
NH, HD = 16, 64
SCALE = 1.0 / float(np.sqrt(HD))  # 0.125
NCORES = 8
HC = H // 128  # 8 contraction chunks of 128

F32R = mybir.dt.float32r
F32 = mybir.dt.float32
BF16 = mybir.dt.bfloat16
EXP = mybir.ActivationFunctionType.Exp

_CACHE = {}


def _emit(nc, tc, ctx, aps, loop_k=None):
    hT, wqT, wkT, wvT, bq_d, bk_d, bv_d, ones_d, outT = aps

    def r(ap):
        return ap

    const = ctx.enter_context(tc.tile_pool(name="const", bufs=1))
    wv_pool = ctx.enter_context(tc.tile_pool(name="wv", bufs=1))
    wqk_pool = ctx.enter_context(tc.tile_pool(name="wqk", bufs=2))
    qk_pool = ctx.enter_context(tc.tile_pool(name="qk", bufs=2))
    att_pool = ctx.enter_context(tc.tile_pool(name="att", bufs=4))
    ctx_pool = ctx.enter_context(tc.tile_pool(name="ctxsb", bufs=4))
    den_pool = ctx.enter_context(tc.tile_pool(name="den", bufs=4))
    bc_pool = ctx.enter_context(tc.tile_pool(name="bc", bufs=4))
    proj_ps = ctx.enter_context(tc.tile_pool(name="proj_ps", bufs=2, space="PSUM"))
    sc_ps = ctx.enter_context(tc.tile_pool(name="sc_ps", bufs=4, space="PSUM"))
    ctx_ps = ctx.enter_context(tc.tile_pool(name="ctx_ps", bufs=2, space="PSUM"))

    if loop_k is not None:
        with tc.For_i(0, loop_k, 1):
            _emit_body(nc, tc, aps, locals_pools=(const, wv_pool, wqk_pool, qk_pool,
                att_pool, ctx_pool, den_pool, bc_pool, proj_ps, sc_ps, ctx_ps))
    else:
        _emit_body(nc, tc, aps, locals_pools=(const, wv_pool, wqk_pool, qk_pool,
            att_pool, ctx_pool, den_pool, bc_pool, proj_ps, sc_ps, ctx_ps))


def _emit_body(nc, tc, aps, locals_pools):
    hT, wqT, wkT, wvT, bq_d, bk_d, bv_d, ones_d, outT = aps
    (const, wv_pool, wqk_pool, qk_pool, att_pool, ctx_pool, den_pool, bc_pool,
     proj_ps, sc_ps, ctx_ps) = locals_pools

    def r(ap):
        return ap

    # ---- constants / big inputs ----
    hT_sb = const.tile([128, HC, L], F32R)
    nc.sync.dma_start(out=hT_sb[:], in_=hT.rearrange("(hc p) i -> p hc i", p=128))
    ones_i = const.tile([1, L], F32R)
    nc.sync.dma_start(out=ones_i[:], in_=ones_d)
    bqs = const.tile([1, H], F32R)
    bks = const.tile([1, H], F32R)
    bvs = const.tile([1, H], F32R)
    nc.sync.dma_start(out=bqs[:], in_=bq_d)
    nc.sync.dma_start(out=bks[:], in_=bk_d)
    nc.sync.dma_start(out=bvs[:], in_=bv_d)
    # vhat[p, jc, head, 0:64] = v, [.., 64] = 1.0 (ones column for denominators)
    vhat = const.tile([128, HC, NH, HD + 1], BF16)
    nc.vector.memset(vhat[:], 1.0)

    # ---- V projection ----
    for dvc in range(2):
        wv_sb = wv_pool.tile([128, HC, 512], F32R, tag="wv")
        nc.sync.dma_start(
            out=wv_sb[:],
            in_=wvT.rearrange("(hc p) d -> p hc d", p=128)[
                :, :, dvc * 512 : (dvc + 1) * 512
            ],
        )
        for jc in range(HC):
            ps = proj_ps.tile([128, 512], F32, tag="proj")
            jsl = slice(jc * 128, (jc + 1) * 128)
            for hc in range(HC):
                nc.tensor.matmul(
                    ps[:], r(hT_sb[:, hc, jsl]), r(wv_sb[:, hc, :]),
                    start=(hc == 0), stop=False,
                )
            nc.tensor.matmul(
                ps[:], r(ones_i[0:1, jsl]),
                r(bvs[0:1, dvc * 512 : (dvc + 1) * 512]),
                start=False, stop=True,
            )
            nc.vector.tensor_copy(
                vhat[:, jc, dvc * 8 : (dvc + 1) * 8, 0:HD],
                ps[:].rearrange("p (h d) -> p h d", d=HD),
            )

    qk_tiles = {}
    att_tiles = {}

    def proj_gen(c):
        """Q/K projection for pair c -> qT/kT [128, L] f32r. Yields 8 times."""
        csl = slice(c * 128, (c + 1) * 128)
        wq_sb = wqk_pool.tile([128, HC, 128], F32R, tag="wq")
        nc.sync.dma_start(
            out=wq_sb[:], in_=wqT.rearrange("(hc p) d -> p hc d", p=128)[:, :, csl]
        )
        wk_sb = wqk_pool.tile([128, HC, 128], F32R, tag="wk")
        nc.sync.dma_start(
            out=wk_sb[:], in_=wkT.rearrange("(hc p) d -> p hc d", p=128)[:, :, csl]
        )
        qT = qk_pool.tile([128, L], F32R, tag="qT")
        kT = qk_pool.tile([128, L], F32R, tag="kT")
        qk_tiles[c] = (qT, kT)
        for dst, w_sb, bias in ((qT, wq_sb, bqs), (kT, wk_sb, bks)):
            for ic in range(2):
                isl = slice(ic * 512, (ic + 1) * 512)
                ps = proj_ps.tile([128, 512], F32, tag="proj")
                for hc in range(HC):
                    nc.tensor.matmul(
                        ps[:], r(w_sb[:, hc, :]), r(hT_sb[:, hc, isl]),
                        start=(hc == 0), stop=False,
                    )
                    if hc == 4:
                        yield
                nc.tensor.matmul(
                    ps[:], r(bias[0:1, csl]), r(ones_i[0:1, isl]),
                    start=False, stop=True,
                )
                nc.vector.tensor_copy(dst[:, isl], ps[:])
                yield

    def qk_gen(c):
        """Scores + exp for pair c. Yields 8 times (once per jc)."""
        qT, kT = qk_tiles.pop(c)
        attA = att_pool.tile([128, HC, L], BF16, tag="att")
        attB = att_pool.tile([128, HC, L], BF16, tag="att")
        att_tiles[c] = (attA, attB)
        for jc in range(HC):
            jsl = slice(jc * 128, (jc + 1) * 128)
            for ic in range(2):
                isl = slice(ic * 512, (ic + 1) * 512)
                psA = sc_ps.tile([128, 512], F32, tag="sc")
                psB = sc_ps.tile([128, 512], F32, tag="sc")
                nc.tensor.matmul(
                    psA[:], r(kT[0:64, jsl]), r(qT[0:64, isl]), start=True, stop=True
                )
                nc.tensor.matmul(
                    psB[:], r(kT[64:128, jsl]), r(qT[64:128, isl]),
                    start=True, stop=True, tile_position=(64, 0),
                )
                nc.scalar.activation(attA[:, jc, isl], psA[:], EXP, scale=SCALE)
                nc.scalar.activation(attB[:, jc, isl], psB[:], EXP, scale=SCALE)
            yield

    def av_gen(c):
        """AV + normalize + output for pair c. Yields 8 times."""
        attA, attB = att_tiles.pop(c)
        for h, att, ic in (
            (2 * c, attA, 0), (2 * c, attA, 1),
            (2 * c + 1, attB, 0), (2 * c + 1, attB, 1),
        ):
            isl = slice(ic * 512, (ic + 1) * 512)
            cps = ctx_ps.tile([HD + 1, 512], F32, tag="ctx")
            for jc in range(HC):
                nc.tensor.matmul(
                    cps[:], vhat[:, jc, h, :], att[:, jc, isl],
                    start=(jc == 0), stop=(jc == HC - 1),
                )
                if jc == 3:
                    yield
            csb = ctx_pool.tile([HD + 1, 512], F32, tag="csb")
            nc.vector.tensor_copy(csb[:], cps[:])
            inv = den_pool.tile([1, 512], F32, tag="inv")
            nc.vector.reciprocal(inv[:], csb[HD : HD + 1, :])
            bc = bc_pool.tile([HD, 512], F32, tag="bc")
            nc.gpsimd.partition_broadcast(bc[:], inv[0:1, :])
            nc.vector.tensor_mul(csb[0:HD, :], csb[0:HD, :], bc[:])
            nc.sync.dma_start(
                out=outT[h * HD : (h + 1) * HD, isl], in_=csb[0:HD, :]
            )
            yield

    # ---- software-pipelined pair loop ----
    NPAIR = NH // 2
    for g in proj_gen(0):
        pass
    for c in range(NPAIR + 1):
        gens = []
        if c < NPAIR:
            gens.append(qk_gen(c))
        if c + 1 < NPAIR:
            gens.append(proj_gen(c + 1))
        if c >= 1:
            gens.append(av_gen(c - 1))
        for _ in range(8):
            for g in gens:
                next(g, None)


def _build(loop_k=None):
    from contextlib import ExitStack

    nc = bacc.Bacc("TRN2", debug=False, num_devices=NCORES)
    hT = nc.dram_tensor("hT", [H, L], F32R, kind="ExternalInput").ap()
    wqT = nc.dram_tensor("wqT", [H, H], F32R, kind="ExternalInput").ap()
    wkT = nc.dram_tensor("wkT", [H, H], F32R, kind="ExternalInput").ap()
    wvT = nc.dram_tensor("wvT", [H, H], F32R, kind="ExternalInput").ap()
    bq_d = nc.dram_tensor("bq", [1, H], F32R, kind="ExternalInput").ap()
    bk_d = nc.dram_tensor("bk", [1, H], F32R, kind="ExternalInput").ap()
    bv_d = nc.dram_tensor("bv", [1, H], F32R, kind="ExternalInput").ap()
    ones_d = nc.dram_tensor("ones", [1, L], F32R, kind="ExternalInput").ap()
    outT = nc.dram_tensor("outT", [H, L], F32, kind="ExternalOutput").ap()
    with tile.TileContext(nc) as tc:
        with ExitStack() as ctx:
            _emit(nc, tc, ctx, (hT, wqT, wkT, wvT, bq_d, bk_d, bv_d, ones_d, outT), loop_k=loop_k)
    nc.compile()
    return nc


def get_nc(loop_k=None):
    key = ("nc", loop_k)
    if key not in _CACHE:
        _CACHE[key] = _build(loop_k=loop_k)
    return _CACHE[key]


def run(hidden_states, Wq, bq, Wk, bk, Wv, bv, loop_k=None, **run_kwargs):
    nc = get_nc(loop_k=loop_k)
    hidden_states = np.asarray(hidden_states, dtype=np.float32)
    wqT = np.ascontiguousarray(np.asarray(Wq, dtype=np.float32).T)
    wkT = np.ascontiguousarray(np.asarray(Wk, dtype=np.float32).T)
    wvT = np.ascontiguousarray(np.asarray(Wv, dtype=np.float32).T)
    bq2 = np.ascontiguousarray(np.asarray(bq, dtype=np.float32).reshape(1, H))
    bk2 = np.ascontiguousarray(np.asarray(bk, dtype=np.float32).reshape(1, H))
    bv2 = np.ascontiguousarray(np.asarray(bv, dtype=np.float32).reshape(1, H))
    ones_row = np.ones((1, L), dtype=np.float32)
    in_maps = []
    for b in range(B):
        in_maps.append(
            {
                "hT": np.ascontiguousarray(hidden_states[b].T),
                "wqT": wqT, "wkT": wkT, "wvT": wvT,
                "bq": bq2, "bk": bk2, "bv": bv2,
                "ones": ones_row,
            }
        )
    res = bass_utils.run_bass_kernel_spmd(
        nc, in_maps, core_ids=list(range(NCORES)), **run_kwargs
    )
    out = np.empty((B, L, H), dtype=np.float32)
    for b in range(B):
        out[b] = res.results[b]["outT"].T
    return out, res


def kernel(hidden_states, Wq, bq, Wk, bk, Wv, bv):
    out, _ = run(hidden_states, Wq, bq, Wk, bk, Wv, bv)
    return out


if __name__ == "__main__":
    rng = np.random.default_rng(0)
    inputs = {
        "hidden_states": rng.standard_normal((B, L, H), dtype=np.float32),
        "Wq": rng.standard_normal((H, H), dtype=np.float32) / 32.0,
        "bq": np.zeros(H, dtype=np.float32),
        "Wk": rng.standard_normal((H, H), dtype=np.float32) / 32.0,
        "bk": np.zeros(H, dtype=np.float32),
        "Wv": rng.standard_normal((H, H), dtype=np.float32) / 32.0,
        "bv": np.zeros(H, dtype=np.float32),
    }
    out = kernel(**inputs)
    print("ran ok", out.shape, out.dtype, float(np.abs(out).max()))


# revision 7
# speedup vs baseline: 3.3251x; 3.3251x over previous
"""BertSelfAttention Trainium2 Bass kernel.

Problem: B=8, L=1024, H=1024, 16 heads x 64 dim, fp32.
Sharding: data-parallel over batch -- one batch element per NeuronCore (8 cores).

Per-core algorithm (everything in "transposed" layout; host transposes in/out):
  inputs:  hT = hidden[b].T  [H, L] f32 (fed to PE as float32r)
           wqT/wkT/wvT = W.T [H, H] f32r, biases [1, H]
  1. v[j, dv] = sum_h hT[h, j] * wvT[h, dv] + bv   (PE, f32r)
       stored as vhat[j, head, 0:64] bf16 with vhat[.., 64] = 1.0 (ones column)
  2. per head-pair c (heads 2c, 2c+1 live in partitions 0:64 / 64:128 of chunk c):
       qT[dq, i], kT[dq, i]  (PE, f32r; bias via K=1 matmul with ones row)
       scoresT[j, i] = kT.T-slice @ qT-slice  -- two K=64 matmuls packed in the
         128-row PE array via tile_position=(64, 0)
       attnT = exp(SCALE * scoresT)  (ACT, PSUM->SBUF, bf16 out; no max-subtraction:
         scores ~ N(0,1), |s|<~6, exact in fp32)
       ctxT[d, i] (+ den in row 64) = vhat.T @ attnT  (PE, bf16, K=1024 accumulated;
         ones column of vhat yields softmax denominator for free)
       ctx = ctxT * (1/den) broadcast  (DVE recip + GpSimd partition_broadcast + DVE mul)
  Emission is software-pipelined: QK/exp of pair c interleaves with projections of
  pair c+1 and AV of pair c-1 so ACT exp time hides under PE work.

Output outT [H, L] per core; host takes outT.T -> ctx[b] [L, H].
"""

import numpy as np

import concourse.bacc as bacc
import concourse.mybir as mybir
import concourse.tile as tile
from concourse import bass_utils

B, L, H = 8, 1024, 1024
NH, HD = 16, 64
SCALE = 1.0 / float(np.sqrt(HD))  # 0.125
NCORES = 8
HC = H // 128  # 8 contraction chunks of 128

F32R = mybir.dt.float32r
F32 = mybir.dt.float32
BF16 = mybir.dt.bfloat16
EXP = mybir.ActivationFunctionType.Exp

_CACHE = {}


def _emit(nc, tc, ctx, aps, loop_k=None):
    hT, wqT, wkT, wvT, bq_d, bk_d, bv_d, ones_d, outT = aps

    def r(ap):
        return ap

    const = ctx.enter_context(tc.tile_pool(name="const", bufs=1))
    wv_pool = ctx.enter_context(tc.tile_pool(name="wv", bufs=1))
    wqk_pool = ctx.enter_context(tc.tile_pool(name="wqk", bufs=2))
    qk_pool = ctx.enter_context(tc.tile_pool(name="qk", bufs=2))
    att_pool = ctx.enter_context(tc.tile_pool(name="att", bufs=4))
    ctx_pool = ctx.enter_context(tc.tile_pool(name="ctxsb", bufs=4))
    den_pool = ctx.enter_context(tc.tile_pool(name="den", bufs=4))
    bc_pool = ctx.enter_context(tc.tile_pool(name="bc", bufs=4))
    proj_ps = ctx.enter_context(tc.tile_pool(name="proj_ps", bufs=2, space="PSUM"))
    sc_ps = ctx.enter_context(tc.tile_pool(name="sc_ps", bufs=4, space="PSUM"))
    ctx_ps = ctx.enter_context(tc.tile_pool(name="ctx_ps", bufs=2, space="PSUM"))

    if loop_k is not None:
        with tc.For_i(0, loop_k, 1):
            _emit_body(nc, tc, aps, locals_pools=(const, wv_pool, wqk_pool, qk_pool,
                att_pool, ctx_pool, den_pool, bc_pool, proj_ps, sc_ps, ctx_ps))
    else:
        _emit_body(nc, tc, aps, locals_pools=(const, wv_pool, wqk_pool, qk_pool,
            att_pool, ctx_pool, den_pool, bc_pool, proj_ps, sc_ps, ctx_ps))


def _emit_body(nc, tc, aps, locals_pools):
    hT, wqT, wkT, wvT, bq_d, bk_d, bv_d, ones_d, outT = aps
    (const, wv_pool, wqk_pool, qk_pool, att_pool, ctx_pool, den_pool, bc_pool,
     proj_ps, sc_ps, ctx_ps) = locals_pools

    def r(ap):
        return ap

    # ---- constants / big inputs ----
    hT_sb = const.tile([128, HC, L], F32R)
    nc.sync.dma_start(out=hT_sb[:], in_=hT.rearrange("(hc p) i -> p hc i", p=128))
    ones_i = const.tile([1, L], F32R)
    nc.sync.dma_start(out=ones_i[:], in_=ones_d)
    bqs = const.tile([1, H], F32R)
    bks = const.tile([1, H], F32R)
    bvs = const.tile([1, H], F32R)
    nc.sync.dma_start(out=bqs[:], in_=bq_d)
    nc.sync.dma_start(out=bks[:], in_=bk_d)
    nc.sync.dma_start(out=bvs[:], in_=bv_d)
    # vhat[p, jc, head, 0:64] = v, [.., 64] = 1.0 (ones column for denominators)
    vhat = const.tile([128, HC, NH, HD + 1], BF16)
    nc.vector.memset(vhat[:], 1.0)

    # ---- V projection ----
    for dvc in range(2):
        wv_sb = wv_pool.tile([128, HC, 512], F32R, tag="wv")
        nc.sync.dma_start(
            out=wv_sb[:],
            in_=wvT.rearrange("(hc p) d -> p hc d", p=128)[
                :, :, dvc * 512 : (dvc + 1) * 512
            ],
        )
        for jc in range(HC):
            ps = proj_ps.tile([128, 512], F32, tag="proj")
            jsl = slice(jc * 128, (jc + 1) * 128)
            for hc in range(HC):
                nc.tensor.matmul(
                    ps[:], r(hT_sb[:, hc, jsl]), r(wv_sb[:, hc, :]),
                    start=(hc == 0), stop=False,
                )
            nc.tensor.matmul(
                ps[:], r(ones_i[0:1, jsl]),
                r(bvs[0:1, dvc * 512 : (dvc + 1) * 512]),
                start=False, stop=True,
            )
            nc.vector.tensor_copy(
                vhat[:, jc, dvc * 8 : (dvc + 1) * 8, 0:HD],
                ps[:].rearrange("p (h d) -> p h d", d=HD),
            )

    qk_tiles = {}
    att_tiles = {}

    def proj_gen(c):
        """Q/K projection for pair c -> qT/kT [128, L] f32r. Yields 8 times."""
        csl = slice(c * 128, (c + 1) * 128)
        wq_sb = wqk_pool.tile([128, HC, 128], F32R, tag="wq")
        nc.sync.dma_start(
            out=wq_sb[:], in_=wqT.rearrange("(hc p) d -> p hc d", p=128)[:, :, csl]
        )
        wk_sb = wqk_pool.tile([128, HC, 128], F32R, tag="wk")
        nc.sync.dma_start(
            out=wk_sb[:], in_=wkT.rearrange("(hc p) d -> p hc d", p=128)[:, :, csl]
        )
        qT = qk_pool.tile([128, L], F32R, tag="qT")
        kT = qk_pool.tile([128, L], F32R, tag="kT")
        qk_tiles[c] = (qT, kT)
        for dst, w_sb, bias in ((qT, wq_sb, bqs), (kT, wk_sb, bks)):
            for ic in range(2):
                isl = slice(ic * 512, (ic + 1) * 512)
                ps = proj_ps.tile([128, 512], F32, tag="proj")
                for hc in range(HC):
                    nc.tensor.matmul(
                        ps[:], r(w_sb[:, hc, :]), r(hT_sb[:, hc, isl]),
                        start=(hc == 0), stop=False,
                    )
                    if hc == 4:
                        yield
                nc.tensor.matmul(
                    ps[:], r(bias[0:1, csl]), r(ones_i[0:1, isl]),
                    start=False, stop=True,
                )
                nc.vector.tensor_copy(dst[:, isl], ps[:])
                yield

    def qk_gen(c):
        """Scores + exp for pair c. Yields 8 times (once per jc)."""
        qT, kT = qk_tiles.pop(c)
        attA = att_pool.tile([128, HC, L], BF16, tag="att")
        attB = att_pool.tile([128, HC, L], BF16, tag="att")
        att_tiles[c] = (attA, attB)
        for jc in range(HC):
            jsl = slice(jc * 128, (jc + 1) * 128)
            for ic in range(2):
                isl = slice(ic * 512, (ic + 1) * 512)
                psA = sc_ps.tile([128, 512], F32, tag="sc")
                psB = sc_ps.tile([128, 512], F32, tag="sc")
                nc.tensor.matmul(
                    psA[:], r(kT[0:64, jsl]), r(qT[0:64, isl]), start=True, stop=True
                )
                nc.tensor.matmul(
                    psB[:], r(kT[64:128, jsl]), r(qT[64:128, isl]),
                    start=True, stop=True, tile_position=(64, 0),
                )
                nc.scalar.activation(attA[:, jc, isl], psA[:], EXP, scale=SCALE)
                nc.scalar.activation(attB[:, jc, isl], psB[:], EXP, scale=SCALE)
            yield

    def av_gen(c):
        """AV + normalize + output for pair c. Yields 8 times."""
        attA, attB = att_tiles.pop(c)
        for h, att, ic in (
            (2 * c, attA, 0), (2 * c, attA, 1),
            (2 * c + 1, attB, 0), (2 * c + 1, attB, 1),
        ):
            isl = slice(ic * 512, (ic + 1) * 512)
            cps = ctx_ps.tile([HD + 1, 512], F32, tag="ctx")
            for jc in range(HC):
                nc.tensor.matmul(
                    cps[:], vhat[:, jc, h, :], att[:, jc, isl],
                    start=(jc == 0), stop=(jc == HC - 1),
                )
                if jc == 3:
                    yield
            csb = ctx_pool.tile([HD + 1, 512], F32, tag="csb")
            nc.vector.tensor_copy(csb[:], cps[:])
            inv = den_pool.tile([1, 512], F32, tag="inv")
            nc.vector.reciprocal(inv[:], csb[HD : HD + 1, :])
            bc = bc_pool.tile([HD, 512], F32, tag="bc")
            nc.gpsimd.partition_broadcast(bc[:], inv[0:1, :])
            nc.vector.tensor_mul(csb[0:HD, :], csb[0:HD, :], bc[:])
            nc.sync.dma_start(
                out=outT[h * HD : (h + 1) * HD, isl], in_=csb[0:HD, :]
            )
            yield

    # ---- software-pipelined pair loop ----
    NPAIR = NH // 2
    for g in proj_gen(0):
        pass
    for c in range(NPAIR + 1):
        gens = []
        if c < NPAIR:
            gens.append(qk_gen(c))
        if c + 1 < NPAIR:
            gens.append(proj_gen(c + 1))
        if c >= 1:
            gens.append(av_gen(c - 1))
        for _ in range(8):
            for g in gens:
                next(g, None)


def _build(loop_k=None):
    from contextlib import ExitStack

    nc = bacc.Bacc("TRN2", debug=False, num_devices=NCORES)
    hT = nc.dram_tensor("hT", [H, L], F32R, kind="ExternalInput").ap()
    wqT = nc.dram_tensor("wqT", [H, H], F32R, kind="ExternalInput").ap()
    wkT = nc.dram_tensor("wkT", [H, H], F32R, kind="ExternalInput").ap()
    wvT = nc.dram_tensor("wvT", [H, H], F32R, kind="ExternalInput").ap()
    bq_d = nc.dram_tensor("bq", [1, H], F32R, kind="ExternalInput").ap()
    bk_d = nc.dram_tensor("bk", [1, H], F32R, kind="ExternalInput").ap()
    bv_d = nc.dram_tensor("bv", [1, H], F32R, kind="ExternalInput").ap()
    ones_d = nc.dram_tensor("ones", [1, L], F32R, kind="ExternalInput").ap()
    outT = nc.dram_tensor("outT", [H, L], F32, kind="ExternalOutput").ap()
    with tile.TileContext(nc) as tc:
        with ExitStack() as ctx:
            _emit(nc, tc, ctx, (hT, wqT, wkT, wvT, bq_d, bk_d, bv_d, ones_d, outT), loop_k=loop_k)
    nc.compile()
    return nc


def get_nc(loop_k=None):
    key = ("nc", loop_k)
    if key not in _CACHE:
        _CACHE[key] = _build(loop_k=loop_k)
    return _CACHE[key]


def run(hidden_states, Wq, bq, Wk, bk, Wv, bv, loop_k=None, **run_kwargs):
    nc = get_nc(loop_k=loop_k)
    hidden_states = np.asarray(hidden_states, dtype=np.float32)
    wqT = np.ascontiguousarray(np.asarray(Wq, dtype=np.float32).T)
    wkT = np.ascontiguousarray(np.asarray(Wk, dtype=np.float32).T)
    wvT = np.ascontiguousarray(np.asarray(Wv, dtype=np.float32).T)
    bq2 = np.ascontiguousarray(np.asarray(bq, dtype=np.float32).reshape(1, H))
    bk2 = np.ascontiguousarray(np.asarray(bk, dtype=np.float32).reshape(1, H))
    bv2 = np.ascontiguousarray(np.asarray(bv, dtype=np.float32).reshape(1, H))
    ones_row = np.ones((1, L), dtype=np.float32)
    in_maps = []
    for b in range(B):
        in_maps.append(
            {
                "hT": np.ascontiguousarray(hidden_states[b].T),
                "wqT": wqT, "wkT": wkT, "wvT": wvT,
                "bq": bq2, "bk": bk2, "bv": bv2,
                "ones": ones_row,
            }
        )
    res = bass_utils.run_bass_kernel_spmd(
        nc, in_maps, core_ids=list(range(NCORES)), **run_kwargs
    )
    out = np.empty((B, L, H), dtype=np.float32)
    for b in range(B):
        out[b] = res.results[b]["outT"].T
    return out, res


def kernel(hidden_states, Wq, bq, Wk, bk, Wv, bv):
    out, _ = run(hidden_states, Wq, bq, Wk, bk, Wv, bv)
    return out


# ---------------- fast cached-executable path (for benchmarking) ----------------

def _make_exec(loop_k=None, donate=True):
    """Build a cached jitted shard_map executable for the kernel NEFF."""
    import jax
    import numpy as _np
    from jax.experimental.shard_map import shard_map
    from jax.sharding import Mesh, PartitionSpec
    import concourse.mybir as _mybir
    from concourse import bass2jax as b2j

    nc = get_nc(loop_k=loop_k)
    b2j.install_neuronx_cc_hook()
    partition_name = nc.partition_id_tensor.name if nc.partition_id_tensor else None
    in_names, out_names, out_avals, zero_outs = [], [], [], []
    for alloc in nc.m.functions[0].allocations:
        if not isinstance(alloc, _mybir.MemoryLocationSet):
            continue
        name = alloc.memorylocations[0].name
        if alloc.kind == "ExternalInput":
            if name != partition_name:
                in_names.append(name)
        elif alloc.kind == "ExternalOutput":
            shape = tuple(alloc.tensor_shape)
            dtype = _mybir.dt.np(alloc.dtype)
            out_names.append(name)
            out_avals.append(jax.core.ShapedArray(shape, dtype))
            zero_outs.append(_np.zeros(shape, dtype))
    n_params = len(in_names)
    n_outs = len(out_avals)
    all_in_names = list(in_names) + list(out_names)
    if partition_name is not None:
        all_in_names.append(partition_name)
    donate_idx = tuple(range(n_params, n_params + n_outs))

    def _body(*args):
        operands = list(args)
        if partition_name is not None:
            operands.append(b2j.partition_id_tensor())
        outs = b2j._bass_exec_p.bind(
            *operands,
            out_avals=tuple(out_avals),
            in_names=tuple(all_in_names),
            out_names=tuple(out_names),
            lowering_input_output_aliases=(),
            sim_require_finite=True,
            sim_require_nnan=True,
            nc=nc,
        )
        return tuple(outs)

    devices = jax.devices()[:NCORES]
    mesh = Mesh(np.asarray(devices), ("core",))
    in_specs = (PartitionSpec("core"),) * (n_params + n_outs)
    out_specs = (PartitionSpec("core"),) * n_outs
    sharded = jax.jit(
        shard_map(_body, mesh=mesh, in_specs=in_specs, out_specs=out_specs,
                  check_rep=False),
        donate_argnums=(donate_idx if donate else ()), keep_unused=True,
    )
    return sharded, in_names, out_names, zero_outs


def get_exec(loop_k=None, donate=True):
    key = ("exec", loop_k, donate)
    if key not in _CACHE:
        _CACHE[key] = _make_exec(loop_k=loop_k, donate=donate)
    return _CACHE[key]


def prep_inputs(hidden_states, Wq, bq, Wk, bk, Wv, bv):
    """Host-side marshalling -> dict of per-core-stacked global arrays."""
    hidden_states = np.asarray(hidden_states, dtype=np.float32)
    wqT = np.ascontiguousarray(np.asarray(Wq, dtype=np.float32).T)
    wkT = np.ascontiguousarray(np.asarray(Wk, dtype=np.float32).T)
    wvT = np.ascontiguousarray(np.asarray(Wv, dtype=np.float32).T)
    bq2 = np.asarray(bq, dtype=np.float32).reshape(1, H)
    bk2 = np.asarray(bk, dtype=np.float32).reshape(1, H)
    bv2 = np.asarray(bv, dtype=np.float32).reshape(1, H)
    ones_row = np.ones((1, L), dtype=np.float32)
    hT_all = np.ascontiguousarray(
        hidden_states.transpose(0, 2, 1).reshape(B * H, L)
    )
    return {
        "hT": hT_all,
        "wqT": np.concatenate([wqT] * B, axis=0),
        "wkT": np.concatenate([wkT] * B, axis=0),
        "wvT": np.concatenate([wvT] * B, axis=0),
        "bq": np.concatenate([bq2] * B, axis=0),
        "bk": np.concatenate([bk2] * B, axis=0),
        "bv": np.concatenate([bv2] * B, axis=0),
        "ones": np.concatenate([ones_row] * B, axis=0),
    }


def run_fast(inputs_concat, loop_k=None, device_inputs=None):
    """Execute via the cached jitted fn. Returns (out [B,L,H], device_inputs)."""
    import jax

    sharded, in_names, out_names, zero_outs = get_exec(loop_k=loop_k)
    if device_inputs is None:
        device_inputs = [jax.device_put(inputs_concat[n]) for n in in_names]
        for a in device_inputs:
            a.block_until_ready()
    zeros = [np.zeros((NCORES * z.shape[0], *z.shape[1:]), z.dtype)
             for z in zero_outs]
    out_arrs = sharded(*device_inputs, *zeros)
    jax.block_until_ready(out_arrs)
    outT_all = np.asarray(out_arrs[0]).reshape(NCORES, H, L)
    out = np.empty((B, L, H), dtype=np.float32)
    for b in range(B):
        out[b] = outT_all[b].T
    return out, device_inputs


if __name__ == "__main__":
    rng = np.random.default_rng(0)
    inputs = {
        "hidden_states": rng.standard_normal((B, L, H), dtype=np.float32),
        "Wq": rng.standard_normal((H, H), dtype=np.float32) / 32.0,
        "bq": np.zeros(H, dtype=np.float32),
        "Wk": rng.standard_normal((H, H), dtype=np.float32) / 32.0,
        "bk": np.zeros(H, dtype=np.float32),
        "Wv": rng.standard_normal((H, H), dtype=np.float32) / 32.0,
        "bv": np.zeros(H, dtype=np.float32),
    }
    out = kernel(**inputs)
    print("ran ok", out.shape, out.dtype, float(np.abs(out).max()))
